# revision 10
# baseline (speedup 1.0000x reference)
"""MoE MiniGPT block kernel for 8 TRN2 NeuronCores (expert-parallel, top-2 sparse).

Strategy:
  - Host computes only integer routing decisions (which tokens are in each
    expert's top-2 set) and reshapes/shards data. All output-value arithmetic
    (router softmax, top-2 renormalized combine weights, FFN with exact GELU,
    aux-loss token statistics) runs on device.
  - Core e receives expert e's weights plus the tokens routed to it (padded to
    a shared capacity C, transposed to [D, C] for the matmul layout).
  - Device per core: router logits -> softmax -> combine weight for this
    expert; FFN y = (gelu(x@w1+b1) @ w2 + b2) * combine; per-core partial sums
    for load-balancing usage and router-z loss (each token is counted on
    exactly its TOP_K=2 cores, so host scales the summed partials by 0.5).
  - Host scatter-adds the two per-expert outputs per token and assembles the
    two scalar losses from device partials.

Matmuls run as float32r (full PE rate); the tiny router/reduction matmuls run
as float32 for precision.
"""

import os

import numpy as np

import concourse.bass as bass
import concourse.mybir as mybir
import concourse.tile as tile
from concourse import bacc
from concourse.bass_utils import run_bass_kernel_spmd

F32 = mybir.dt.float32
F32R = mybir.dt.float32r

E = 8
TOP_K = 2
D = 768
F = 3072
KD = D // 128  # 6
KF = F // 128  # 24
TB = 384  # token block (3 sub-tiles of 128)
LB_W = 0.01
Z_W = 0.001

AF = mybir.ActivationFunctionType
ALU = mybir.AluOpType
AX = mybir.AxisListType

_MM_DT = F32R if os.environ.get("MOE_MM_DT", "f32r") == "f32r" else F32


def build_moe_program(C: int):
    """Trace the SPMD Bass/Tile program for capacity C (multiple of TB)."""
    assert C % TB == 0
    nts = C // 128
    nb = C // TB
    phases = os.environ.get("MOE_PHASE", "12")
    p1cut = int(os.environ.get("MOE_P1CUT", "99"))

    nc = bacc.Bacc(None, target_bir_lowering=False)

    xgT = nc.dram_tensor("xgT", [D, C], _MM_DT, kind="ExternalInput")
    Wr = nc.dram_tensor("Wr", [D, E], _MM_DT, kind="ExternalInput")
    w1 = nc.dram_tensor("w1", [D, F], _MM_DT, kind="ExternalInput")
    b1T = nc.dram_tensor("b1T", [128, KF], F32, kind="ExternalInput")
    w2 = nc.dram_tensor("w2", [F, D], _MM_DT, kind="ExternalInput")
    b2r = nc.dram_tensor("b2r", [1, D], _MM_DT, kind="ExternalInput")
    sel = nc.dram_tensor("sel", [128, E], F32, kind="ExternalInput")
    maskc = nc.dram_tensor("maskc", [128, nts], F32, kind="ExternalInput")
    onescol = nc.dram_tensor("onescol", [128, 1], F32, kind="ExternalInput")
    onesrow = nc.dram_tensor("onesrow", [1, 128], _MM_DT, kind="ExternalInput")
    yg = nc.dram_tensor("yg", [C, D], F32, kind="ExternalOutput")
    partials = nc.dram_tensor("partials", [1, E + 1], F32, kind="ExternalOutput")

    with tile.TileContext(nc) as tc:
        with (
            tc.tile_pool(name="weights", bufs=1) as wp,
            tc.tile_pool(name="consts", bufs=1) as cp,
        ):
            w1_sb = wp.tile([128, KD, F], _MM_DT, tag="w1")
            nc.sync.dma_start(w1_sb[:], w1[:].rearrange("(k p) f -> p k f", p=128))
            w2_sb = wp.tile([128, KF, D], _MM_DT, tag="w2")
            nc.sync.dma_start(w2_sb[:], w2[:].rearrange("(k p) d -> p k d", p=128))

            wr_sb = cp.tile([128, KD, E], _MM_DT, tag="wr")
            nc.sync.dma_start(wr_sb[:], Wr[:].rearrange("(k p) e -> p k e", p=128))
            b1_sb = cp.tile([128, KF], F32, tag="b1")
            nc.sync.dma_start(b1_sb[:], b1T[:])
            b2_sb = cp.tile([1, D], _MM_DT, tag="b2")
            nc.sync.dma_start(b2_sb[:], b2r[:])
            sel_sb = cp.tile([128, E], F32, tag="sel")
            nc.sync.dma_start(sel_sb[:], sel[:])
            mask_sb = cp.tile([128, nts], F32, tag="mask")
            nc.sync.dma_start(mask_sb[:], maskc[:])
            ones_c = cp.tile([128, 1], F32, tag="onesc")
            nc.sync.dma_start(ones_c[:], onescol[:])
            ones_r = cp.tile([1, 128], _MM_DT, tag="onesr")
            nc.sync.dma_start(ones_r[:], onesrow[:])
            comb_all = cp.tile([128, nts], F32, tag="comb")
            ucols = cp.tile([128, nts, E], F32, tag="ucols")
            zcols = cp.tile([128, nts], F32, tag="zcols")
            stag = cp.tile([128, E + 1], F32, tag="stag")

            # ---------------- Phase 1: router / combine / loss partials ----
            with (
                tc.tile_pool(name="p1x", bufs=3) as p1x,
                tc.tile_pool(name="p1ps", bufs=4, space="PSUM") as p1ps,
                tc.tile_pool(name="p1s", bufs=4) as p1s,
                tc.tile_pool(name="p1w", bufs=4) as p1w,
            ):
                xgT_r = xgT[:].rearrange("(k p) c -> p k c", p=128)
                for ts in range(nts if "1" in phases else 0):
                    xr = p1x.tile([128, KD, 128], _MM_DT, tag="xr")
                    nc.sync.dma_start(xr[:], xgT_r[:, :, ts * 128:(ts + 1) * 128])
                    if p1cut < 2: continue
                    lg = p1ps.tile([128, E], F32, tag="lg")
                    for k in range(KD):
                        nc.tensor.matmul(
                            lg[:],
                            lhsT=xr[:, k, :],
                            rhs=wr_sb[:, k, :],
                            start=(k == 0),
                            stop=(k == KD - 1),
                        )
                    if p1cut < 3: continue
                    negmax = p1s.tile([128, 1], F32, tag="negmax")
                    nc.vector.tensor_reduce(
                        negmax[:], lg[:], axis=AX.X, op=ALU.max, negate=True
                    )
                    if p1cut < 4: continue
                    expt = p1w.tile([128, E], F32, tag="expt")
                    S = p1s.tile([128, 1], F32, tag="S")
                    nc.scalar.activation(
                        expt[:], lg[:], AF.Exp, bias=negmax[:], accum_out=S[:]
                    )
                    if p1cut < 5: continue
                    rS = p1s.tile([128, 1], F32, tag="rS")
                    nc.vector.reciprocal(rS[:], S[:])
                    probs = p1w.tile([128, E], F32, tag="probs")
                    nc.vector.tensor_scalar_mul(probs[:], expt[:], rS[:])
                    if p1cut < 6: continue
                    m1 = p1s.tile([128, 1], F32, tag="m1")
                    nc.vector.tensor_reduce(m1[:], probs[:], axis=AX.X, op=ALU.max)
                    # mask out the top-1 entry, then take max again for top-2
                    istop = p1w.tile([128, E], F32, tag="istop")
                    nc.vector.tensor_scalar(istop[:], probs[:], m1[:], None, op0=ALU.is_ge)
                    masked = p1w.tile([128, E], F32, tag="masked")
                    nc.vector.scalar_tensor_tensor(
                        masked[:], istop[:], -1e9, probs[:], op0=ALU.mult, op1=ALU.add
                    )
                    m2 = p1s.tile([128, 1], F32, tag="m2")
                    nc.vector.tensor_reduce(m2[:], masked[:], axis=AX.X, op=ALU.max)
                    if p1cut < 7: continue
                    den = p1s.tile([128, 1], F32, tag="den")
                    nc.vector.tensor_tensor(den[:], m1[:], m2[:], op=ALU.add)
                    den2 = p1s.tile([128, 1], F32, tag="den2")
                    nc.vector.tensor_scalar_add(den2[:], den[:], 1e-8)
                    rden = p1s.tile([128, 1], F32, tag="rden")
                    nc.vector.reciprocal(rden[:], den2[:])
                    if p1cut < 8: continue
                    # p_sel = sum(probs * sel); combine = p_sel * rden
                    psel_t = p1w.tile([128, E], F32, tag="pselt")
                    psel = p1s.tile([128, 1], F32, tag="psel")
                    nc.vector.tensor_tensor(psel_t[:], probs[:], sel_sb[:], op=ALU.mult)
                    nc.vector.tensor_reduce(psel[:], psel_t[:], axis=AX.X, op=ALU.add)
                    nc.vector.tensor_tensor(
                        comb_all[:, ts:ts + 1], psel[:], rden[:], op=ALU.mult
                    )
                    if p1cut < 9: continue
                    # loss partials (masked for padding)
                    lse = p1s.tile([128, 1], F32, tag="lse")
                    nc.scalar.activation(lse[:], S[:], AF.Ln)
                    lse2 = p1s.tile([128, 1], F32, tag="lse2")
                    nc.vector.tensor_tensor(lse2[:], lse[:], negmax[:], op=ALU.subtract)
                    lsq = p1s.tile([128, 1], F32, tag="lsq")
                    nc.vector.tensor_tensor(lsq[:], lse2[:], lse2[:], op=ALU.mult)
                    nc.vector.tensor_tensor(
                        zcols[:, ts:ts + 1], lsq[:], mask_sb[:, ts:ts + 1], op=ALU.mult
                    )
                    nc.vector.tensor_scalar_mul(
                        ucols[:, ts, :], probs[:], mask_sb[:, ts:ts + 1]
                    )
                # reduce partials across t_subs, then across partitions
                if "1" in phases and p1cut >= 10:
                    nc.vector.tensor_reduce(
                        stag[:, 0:E],
                        ucols[:].rearrange("p t e -> p e t"),
                        axis=AX.X,
                        op=ALU.add,
                    )
                    nc.vector.tensor_reduce(
                        stag[:, E:E + 1], zcols[:], axis=AX.X, op=ALU.add
                    )
                    pred = p1ps.tile([1, E + 1], F32, tag="pred")
                    nc.tensor.matmul(
                        pred[:], lhsT=ones_c[:], rhs=stag[:], start=True, stop=True
                    )
                    part_sb = p1s.tile([1, E + 1], F32, tag="partsb")
                    nc.scalar.copy(part_sb[:], pred[:])
                    nc.sync.dma_start(partials[:], part_sb[:])

            # ---------------- Phase 2: FFN ---------------------------------
            with (
                tc.tile_pool(name="p2x", bufs=2) as p2x,
                tc.tile_pool(name="p2h", bufs=3) as p2h,
                tc.tile_pool(name="p2y", bufs=3) as p2y,
                tc.tile_pool(name="psh", bufs=2, space="PSUM") as psh,
                tc.tile_pool(name="psy", bufs=3, space="PSUM") as psy,
            ):
                for b in range(nb if "2" in phases else 0):
                    xb = p2x.tile([128, KD, TB], _MM_DT, tag="xb")
                    nc.sync.dma_start(xb[:], xgT_r[:, :, b * TB:(b + 1) * TB])
                    py = []
                    for ts in range(3):
                        py.append(psy.tile([128, D], F32, tag="py", name=f"py{ts}"))
                    for f in range(KF):
                        ph = psh.tile([128, TB], F32, tag="ph")
                        for k in range(KD):
                            nc.tensor.matmul(
                                ph[:],
                                lhsT=(w1_sb[:, k, f * 128:(f + 1) * 128]),
                                rhs=(xb[:, k, :]),
                                start=(k == 0),
                                stop=(k == KD - 1),
                            )
                        hs = p2h.tile([128, TB], _MM_DT, tag="hs")
                        nc.scalar.activation(
                            hs[:], ph[:], AF.Gelu, bias=b1_sb[:, f:f + 1]
                        )
                        for ts in range(3):
                            nc.tensor.matmul(
                                py[ts][:, 0:512],
                                lhsT=(hs[:, ts * 128:(ts + 1) * 128]),
                                rhs=(w2_sb[:, f, 0:512]),
                                start=(f == 0),
                                stop=False,
                            )
                            nc.tensor.matmul(
                                py[ts][:, 512:D],
                                lhsT=(hs[:, ts * 128:(ts + 1) * 128]),
                                rhs=(w2_sb[:, f, 512:D]),
                                start=(f == 0),
                                stop=False,
                            )
                    for ts in range(3):
                        # bias add as rank-1 matmul: py += ones^T(1x128) @ b2(1xD)
                        nc.tensor.matmul(
                            py[ts][:, 0:512],
                            lhsT=(ones_r[:]),
                            rhs=(b2_sb[:, 0:512]),
                            start=False,
                            stop=True,
                        )
                        nc.tensor.matmul(
                            py[ts][:, 512:D],
                            lhsT=(ones_r[:]),
                            rhs=(b2_sb[:, 512:D]),
                            start=False,
                            stop=True,
                        )
                        gts = b * 3 + ts
                        ysb = p2y.tile([128, D], F32, tag="ysb")
                        nc.scalar.activation(
                            ysb[:], py[ts][:], AF.Copy,
                            scale=comb_all[:, gts:gts + 1],
                        )
                        nc.sync.dma_start(
                            yg[b * TB + ts * 128:b * TB + (ts + 1) * 128, :], ysb[:]
                        )
    nc.compile()
    return nc


# ---------------------------------------------------------------------------
# Host side
# ---------------------------------------------------------------------------

def _route(x2d: np.ndarray, Wr: np.ndarray):
    """Top-2 routing decisions (integer only; values recomputed on device)."""
    logits = x2d @ Wr  # [N, E] fp32
    lmax = logits.max(-1, keepdims=True)
    p = np.exp(logits - lmax)
    p /= p.sum(-1, keepdims=True)
    top2 = np.argsort(-p, axis=-1, kind="stable")[:, :TOP_K]  # [N, 2]
    return top2


def _prep_core_inputs(x2d, Wr, w1, b1, w2, b2, top2):
    n_tok = x2d.shape[0]
    lists = [np.nonzero((top2 == e).any(-1))[0] for e in range(E)]
    cmax = max(len(l) for l in lists)
    C = max(TB, ((cmax + TB - 1) // TB) * TB)
    nts = C // 128
    in_maps = []
    for e in range(E):
        idx = lists[e]
        cnt = len(idx)
        xg = np.zeros((C, D), np.float32)
        xg[:cnt] = x2d[idx]
        mask = np.zeros((128, nts), np.float32)
        flat = np.arange(C) < cnt
        mask[:, :] = flat.reshape(nts, 128).T
        sel = np.zeros((128, E), np.float32)
        sel[:, e] = 1.0
        in_maps.append({
            "xgT": np.ascontiguousarray(xg.T),
            "Wr": np.ascontiguousarray(Wr),
            "w1": np.ascontiguousarray(w1[e]),
            "b1T": np.ascontiguousarray(b1[e].reshape(KF, 128).T),
            "w2": np.ascontiguousarray(w2[e]),
            "b2r": np.ascontiguousarray(b2[e][None, :]),
            "sel": sel,
            "maskc": mask,
            "onescol": np.ones((128, 1), np.float32),
            "onesrow": np.ones((1, 128), np.float32),
        })
    return in_maps, lists, C


_PROGRAM_CACHE: dict[int, object] = {}


def _get_program(C: int):
    if os.environ.get("MOE_PHASE"):
        return build_moe_program(C)
    if C not in _PROGRAM_CACHE:
        _PROGRAM_CACHE[C] = build_moe_program(C)
    return _PROGRAM_CACHE[C]


def _maybe_install_trace_shim():
    """antenv.axon_hooks is absent in this image; recreate it from trn_boot so
    trace=True can capture NTFF profiles through the axon .so."""
    import sys
    import types

    if "antenv.axon_hooks" in sys.modules:
        return
    try:
        from trn_agent_boot.trn_boot import _ntff_profile_via_ctypes

        hook = _ntff_profile_via_ctypes("/opt/axon/libaxon_pjrt.so")
    except Exception:
        return
    mod = types.ModuleType("antenv.axon_hooks")
    mod.get_axon_ntff_profile_hook = lambda: hook
    mod.set_axon_ntff_profile_hook = lambda h: None
    sys.modules["antenv.axon_hooks"] = mod


def run_moe(inputs: dict, trace: bool = False):
    """Run the kernel; returns ((out, lb_loss, z_loss), BassKernelResults)."""
    x = np.asarray(inputs["x"], np.float32)
    Wr = np.asarray(inputs["Wr"], np.float32)
    w1 = np.asarray(inputs["w1"], np.float32)
    b1 = np.asarray(inputs["b1"], np.float32)
    w2 = np.asarray(inputs["w2"], np.float32)
    b2 = np.asarray(inputs["b2"], np.float32)

    B, S, _ = x.shape
    x2d = np.ascontiguousarray(x.reshape(B * S, D))
    top2 = _route(x2d, Wr)
    in_maps, lists, C = _prep_core_inputs(x2d, Wr, w1, b1, w2, b2, top2)
    nc = _get_program(C)

    if trace:
        _maybe_install_trace_shim()
    res = run_bass_kernel_spmd(
        nc, in_maps, core_ids=list(range(E)), trace=trace,
        trace_cores=list(range(E)) if trace else None, stitch_traces=False,
    )

    out2d = np.zeros((B * S, D), np.float32)
    usage_sum = np.zeros(E, np.float32)
    z_sum = np.float32(0.0)
    for e in range(E):
        r = res.results[e]
        idx = lists[e]
        out2d[idx] += r["yg"][: len(idx)]
        usage_sum += r["partials"][0, :E]
        z_sum += r["partials"][0, E]

    n_tok = np.float32(B * S)
    usage = usage_sum * np.float32(0.5) / n_tok
    mu = usage.mean(dtype=np.float32)
    var = ((usage - mu) ** 2).mean(dtype=np.float32)
    lb = var / (mu * mu + np.float32(1e-8)) * np.float32(E) * np.float32(LB_W)
    z = np.float32(0.5) * z_sum / n_tok * np.float32(Z_W)

    out = out2d.reshape(B, S, D)
    return (out, np.float32(lb), np.float32(z)), res


def kernel(**inputs):
    result, _ = run_moe(inputs, trace=bool(int(os.environ.get("MOE_TRACE", "0"))))
    return result


# revision 11
# speedup vs baseline: 1.1609x; 1.1609x over previous
"""MoE MiniGPT block kernel for 8 TRN2 NeuronCores (expert-parallel, top-2 sparse).

Strategy:
  - Host computes only integer routing decisions (which tokens are in each
    expert's top-2 set) and reshapes/shards data. All output-value arithmetic
    (router softmax, top-2 renormalized combine weights, FFN with exact GELU,
    aux-loss token statistics) runs on device.
  - Core e receives expert e's weights plus the tokens routed to it (padded to
    a shared capacity C, transposed to [D, C] for the matmul layout).
  - Device per core: router logits -> softmax -> combine weight for this
    expert; FFN y = (gelu(x@w1+b1) @ w2 + b2) * combine; per-core partial sums
    for load-balancing usage and router-z loss (each token is counted on
    exactly its TOP_K=2 cores, so host scales the summed partials by 0.5).
  - Host scatter-adds the two per-expert outputs per token and assembles the
    two scalar losses from device partials.

Matmuls run as float32r (full PE rate); the tiny router/reduction matmuls run
as float32 for precision.
"""

import os

import numpy as np

import concourse.bass as bass
import concourse.mybir as mybir
import concourse.tile as tile
from concourse import bacc
from concourse.bass_utils import run_bass_kernel_spmd

F32 = mybir.dt.float32
F32R = mybir.dt.float32r

E = 8
TOP_K = 2
D = 768
F = 3072
KD = D // 128  # 6
KF = F // 128  # 24
TB = 384  # token block (3 sub-tiles of 128)
LB_W = 0.01
Z_W = 0.001

AF = mybir.ActivationFunctionType
ALU = mybir.AluOpType
AX = mybir.AxisListType

_MM_DT = F32R if os.environ.get("MOE_MM_DT", "f32r") == "f32r" else F32


def build_moe_program(C: int):
    """Trace the SPMD Bass/Tile program for capacity C (multiple of TB)."""
    assert C % TB == 0
    nts = C // 128
    nb = C // TB

    nc = bacc.Bacc(None, target_bir_lowering=False)

    xgT = nc.dram_tensor("xgT", [D, C], _MM_DT, kind="ExternalInput")
    Wr = nc.dram_tensor("Wr", [D, E], _MM_DT, kind="ExternalInput")
    w1 = nc.dram_tensor("w1", [D, F], _MM_DT, kind="ExternalInput")
    b1T = nc.dram_tensor("b1T", [128, KF], F32, kind="ExternalInput")
    w2 = nc.dram_tensor("w2", [F, D], _MM_DT, kind="ExternalInput")
    b2r = nc.dram_tensor("b2r", [1, D], _MM_DT, kind="ExternalInput")
    sel = nc.dram_tensor("sel", [128, E], F32, kind="ExternalInput")
    maskc = nc.dram_tensor("maskc", [128, nts], F32, kind="ExternalInput")
    onescol = nc.dram_tensor("onescol", [128, 1], F32, kind="ExternalInput")
    onesrow = nc.dram_tensor("onesrow", [1, 128], _MM_DT, kind="ExternalInput")
    yg = nc.dram_tensor("yg", [C, D], F32, kind="ExternalOutput")
    partials = nc.dram_tensor("partials", [1, E + 1], F32, kind="ExternalOutput")

    WG = 4  # weight DMA split (per-queue pipelining granularity)

    with tile.TileContext(nc) as tc:
        with (
            tc.tile_pool(name="weights", bufs=1) as wp,
            tc.tile_pool(name="consts", bufs=1) as cp,
            tc.tile_pool(name="p1s", bufs=4) as p1s,
            tc.tile_pool(name="p1w", bufs=4) as p1w,
            tc.tile_pool(name="p2x", bufs=2) as p2x,
            tc.tile_pool(name="p2h", bufs=3) as p2h,
            tc.tile_pool(name="p2y", bufs=3) as p2y,
            tc.tile_pool(name="psh", bufs=2, space="PSUM") as psh,
            tc.tile_pool(name="psy", bufs=3, space="PSUM") as psy,
        ):
            # Weight loads: w1 on the gpsimd SWDGE queue, w2 on the scalar
            # HWDGE queue, both split into groups so the FFN can start as
            # soon as the first group lands. Everything else uses sync.
            w1_sb = wp.tile([128, KD, F], _MM_DT, tag="w1")
            w1_r = w1[:].rearrange("(k p) f -> p k f", p=128)
            for g in range(WG):
                gs = F // WG
                nc.gpsimd.dma_start(
                    w1_sb[:, :, g * gs:(g + 1) * gs], w1_r[:, :, g * gs:(g + 1) * gs]
                )
            w2_sb = wp.tile([128, KF, D], _MM_DT, tag="w2")
            w2_r = w2[:].rearrange("(k p) d -> p k d", p=128)
            for g in range(WG):
                gs = KF // WG
                nc.scalar.dma_start(
                    w2_sb[:, g * gs:(g + 1) * gs, :], w2_r[:, g * gs:(g + 1) * gs, :]
                )

            wr_sb = cp.tile([128, KD, E], _MM_DT, tag="wr")
            nc.sync.dma_start(wr_sb[:], Wr[:].rearrange("(k p) e -> p k e", p=128))
            b1_sb = cp.tile([128, KF], F32, tag="b1")
            nc.sync.dma_start(b1_sb[:], b1T[:])
            b2_sb = cp.tile([1, D], _MM_DT, tag="b2")
            nc.sync.dma_start(b2_sb[:], b2r[:])
            sel_sb = cp.tile([128, E], F32, tag="sel")
            nc.sync.dma_start(sel_sb[:], sel[:])
            mask_sb = cp.tile([128, nts], F32, tag="mask")
            nc.sync.dma_start(mask_sb[:], maskc[:])
            ones_c = cp.tile([128, 1], F32, tag="onesc")
            nc.sync.dma_start(ones_c[:], onescol[:])
            ones_r = cp.tile([1, 128], _MM_DT, tag="onesr")
            nc.sync.dma_start(ones_r[:], onesrow[:])
            comb_all = cp.tile([128, nts], F32, tag="comb")
            ucols = cp.tile([128, nts, E], F32, tag="ucols")
            scols = cp.tile([128, nts], F32, tag="scols")
            ncols = cp.tile([128, nts], F32, tag="ncols")
            stag = cp.tile([128, E + 1], F32, tag="stag")

            xgT_r = xgT[:].rearrange("(k p) c -> p k c", p=128)
            for b in range(nb):
                xb = p2x.tile([128, KD, TB], _MM_DT, tag="xb")
                nc.sync.dma_start(xb[:], xgT_r[:, :, b * TB:(b + 1) * TB])

                # router / combine / loss stats for this block's 3 sub-tiles
                for tsl in range(3):
                    ts = b * 3 + tsl
                    lg = psh.tile([128, E], F32, tag="ph", name=f"lg{ts}")
                    for k in range(KD):
                        nc.tensor.matmul(
                            lg[:],
                            lhsT=xb[:, k, tsl * 128:(tsl + 1) * 128],
                            rhs=wr_sb[:, k, :],
                            start=(k == 0),
                            stop=(k == KD - 1),
                        )
                    negmax = ncols[:, ts:ts + 1]
                    nc.vector.tensor_reduce(
                        negmax, lg[:], axis=AX.X, op=ALU.max, negate=True
                    )
                    expt = p1w.tile([128, E], F32, tag="expt")
                    nc.scalar.activation(
                        expt[:], lg[:], AF.Exp, bias=negmax,
                        accum_out=scols[:, ts:ts + 1],
                    )
                    rS = p1s.tile([128, 1], F32, tag="rS")
                    nc.vector.reciprocal(rS[:], scols[:, ts:ts + 1])
                    probs = p1w.tile([128, E], F32, tag="probs")
                    nc.vector.tensor_scalar_mul(probs[:], expt[:], rS[:])
                    m1 = p1s.tile([128, 1], F32, tag="m1")
                    nc.vector.tensor_reduce(m1[:], probs[:], axis=AX.X, op=ALU.max)
                    istop = p1w.tile([128, E], F32, tag="istop")
                    nc.vector.tensor_scalar(
                        istop[:], probs[:], m1[:], None, op0=ALU.is_ge
                    )
                    masked = p1w.tile([128, E], F32, tag="masked")
                    nc.vector.scalar_tensor_tensor(
                        masked[:], istop[:], -1e9, probs[:], op0=ALU.mult, op1=ALU.add
                    )
                    m2 = p1s.tile([128, 1], F32, tag="m2")
                    nc.vector.tensor_reduce(m2[:], masked[:], axis=AX.X, op=ALU.max)
                    den = p1s.tile([128, 1], F32, tag="den")
                    nc.vector.tensor_tensor(den[:], m1[:], m2[:], op=ALU.add)
                    den2 = p1s.tile([128, 1], F32, tag="den2")
                    nc.vector.tensor_scalar_add(den2[:], den[:], 1e-8)
                    rden = p1s.tile([128, 1], F32, tag="rden")
                    nc.vector.reciprocal(rden[:], den2[:])
                    psel_t = p1w.tile([128, E], F32, tag="pselt")
                    psel = p1s.tile([128, 1], F32, tag="psel")
                    nc.vector.tensor_tensor(psel_t[:], probs[:], sel_sb[:], op=ALU.mult)
                    nc.vector.tensor_reduce(psel[:], psel_t[:], axis=AX.X, op=ALU.add)
                    nc.vector.tensor_tensor(
                        comb_all[:, ts:ts + 1], psel[:], rden[:], op=ALU.mult
                    )
                    nc.vector.tensor_scalar_mul(
                        ucols[:, ts, :], probs[:], mask_sb[:, ts:ts + 1]
                    )

                # FFN
                py = []
                for ts in range(3):
                    py.append(psy.tile([128, D], F32, tag="py", name=f"py{ts}"))
                for f in range(KF):
                    ph = psh.tile([128, TB], F32, tag="ph")
                    for k in range(KD):
                        nc.tensor.matmul(
                            ph[:],
                            lhsT=(w1_sb[:, k, f * 128:(f + 1) * 128]),
                            rhs=(xb[:, k, :]),
                            start=(k == 0),
                            stop=(k == KD - 1),
                        )
                    hs = p2h.tile([128, TB], _MM_DT, tag="hs")
                    nc.scalar.activation(
                        hs[:], ph[:], AF.Gelu, bias=b1_sb[:, f:f + 1]
                    )
                    for ts in range(3):
                        nc.tensor.matmul(
                            py[ts][:, 0:512],
                            lhsT=(hs[:, ts * 128:(ts + 1) * 128]),
                            rhs=(w2_sb[:, f, 0:512]),
                            start=(f == 0),
                            stop=False,
                        )
                        nc.tensor.matmul(
                            py[ts][:, 512:D],
                            lhsT=(hs[:, ts * 128:(ts + 1) * 128]),
                            rhs=(w2_sb[:, f, 512:D]),
                            start=(f == 0),
                            stop=False,
                        )
                for ts in range(3):
                    # bias add as rank-1 matmul: py += ones^T(1x128) @ b2(1xD)
                    nc.tensor.matmul(
                        py[ts][:, 0:512],
                        lhsT=(ones_r[:]),
                        rhs=(b2_sb[:, 0:512]),
                        start=False,
                        stop=True,
                    )
                    nc.tensor.matmul(
                        py[ts][:, 512:D],
                        lhsT=(ones_r[:]),
                        rhs=(b2_sb[:, 512:D]),
                        start=False,
                        stop=True,
                    )
                    gts = b * 3 + ts
                    ysb = p2y.tile([128, D], F32, tag="ysb")
                    nc.vector.tensor_scalar_mul(
                        ysb[:], py[ts][:], comb_all[:, gts:gts + 1]
                    )
                    nc.sync.dma_start(
                        yg[b * TB + ts * 128:b * TB + (ts + 1) * 128, :], ysb[:]
                    )

            # loss partials: z-columns batched, then partition-reduce
            lnall = p1w.tile([128, nts], F32, tag="lnall")
            nc.scalar.activation(lnall[:], scols[:], AF.Ln)
            lseall = p1w.tile([128, nts], F32, tag="lseall")
            nc.vector.tensor_tensor(lseall[:], lnall[:], ncols[:], op=ALU.subtract)
            lsqall = p1w.tile([128, nts], F32, tag="lsqall")
            nc.vector.tensor_tensor(lsqall[:], lseall[:], lseall[:], op=ALU.mult)
            zcols = p1w.tile([128, nts], F32, tag="zcols")
            nc.vector.tensor_tensor(zcols[:], lsqall[:], mask_sb[:], op=ALU.mult)
            nc.vector.tensor_reduce(
                stag[:, 0:E],
                ucols[:].rearrange("p t e -> p e t"),
                axis=AX.X,
                op=ALU.add,
            )
            nc.vector.tensor_reduce(stag[:, E:E + 1], zcols[:], axis=AX.X, op=ALU.add)
            pred = psh.tile([1, E + 1], F32, tag="ph", name="pred")
            nc.tensor.matmul(pred[:], lhsT=ones_c[:], rhs=stag[:], start=True, stop=True)
            part_sb = p1s.tile([1, E + 1], F32, tag="partsb")
            nc.scalar.copy(part_sb[:], pred[:])
            nc.sync.dma_start(partials[:], part_sb[:])
    nc.compile()
    return nc


# ---------------------------------------------------------------------------
# Host side
# ---------------------------------------------------------------------------

def _route(x2d: np.ndarray, Wr: np.ndarray):
    """Top-2 routing decisions (integer only; values recomputed on device)."""
    logits = x2d @ Wr  # [N, E] fp32
    lmax = logits.max(-1, keepdims=True)
    p = np.exp(logits - lmax)
    p /= p.sum(-1, keepdims=True)
    top2 = np.argsort(-p, axis=-1, kind="stable")[:, :TOP_K]  # [N, 2]
    return top2


def _prep_core_inputs(x2d, Wr, w1, b1, w2, b2, top2):
    n_tok = x2d.shape[0]
    lists = [np.nonzero((top2 == e).any(-1))[0] for e in range(E)]
    cmax = max(len(l) for l in lists)
    C = max(TB, ((cmax + TB - 1) // TB) * TB)
    nts = C // 128
    in_maps = []
    for e in range(E):
        idx = lists[e]
        cnt = len(idx)
        xg = np.zeros((C, D), np.float32)
        xg[:cnt] = x2d[idx]
        mask = np.zeros((128, nts), np.float32)
        flat = np.arange(C) < cnt
        mask[:, :] = flat.reshape(nts, 128).T
        sel = np.zeros((128, E), np.float32)
        sel[:, e] = 1.0
        in_maps.append({
            "xgT": np.ascontiguousarray(xg.T),
            "Wr": np.ascontiguousarray(Wr),
            "w1": np.ascontiguousarray(w1[e]),
            "b1T": np.ascontiguousarray(b1[e].reshape(KF, 128).T),
            "w2": np.ascontiguousarray(w2[e]),
            "b2r": np.ascontiguousarray(b2[e][None, :]),
            "sel": sel,
            "maskc": mask,
            "onescol": np.ones((128, 1), np.float32),
            "onesrow": np.ones((1, 128), np.float32),
        })
    return in_maps, lists, C


_PROGRAM_CACHE: dict[int, object] = {}


def _get_program(C: int):
    if os.environ.get("MOE_PHASE"):
        return build_moe_program(C)
    if C not in _PROGRAM_CACHE:
        _PROGRAM_CACHE[C] = build_moe_program(C)
    return _PROGRAM_CACHE[C]


def _maybe_install_trace_shim():
    """antenv.axon_hooks is absent in this image; recreate it from trn_boot so
    trace=True can capture NTFF profiles through the axon .so."""
    import sys
    import types

    if "antenv.axon_hooks" in sys.modules:
        return
    try:
        from trn_agent_boot.trn_boot import _ntff_profile_via_ctypes

        hook = _ntff_profile_via_ctypes("/opt/axon/libaxon_pjrt.so")
    except Exception:
        return
    mod = types.ModuleType("antenv.axon_hooks")
    mod.get_axon_ntff_profile_hook = lambda: hook
    mod.set_axon_ntff_profile_hook = lambda h: None
    sys.modules["antenv.axon_hooks"] = mod


def run_moe(inputs: dict, trace: bool = False):
    """Run the kernel; returns ((out, lb_loss, z_loss), BassKernelResults)."""
    x = np.asarray(inputs["x"], np.float32)
    Wr = np.asarray(inputs["Wr"], np.float32)
    w1 = np.asarray(inputs["w1"], np.float32)
    b1 = np.asarray(inputs["b1"], np.float32)
    w2 = np.asarray(inputs["w2"], np.float32)
    b2 = np.asarray(inputs["b2"], np.float32)

    B, S, _ = x.shape
    x2d = np.ascontiguousarray(x.reshape(B * S, D))
    top2 = _route(x2d, Wr)
    in_maps, lists, C = _prep_core_inputs(x2d, Wr, w1, b1, w2, b2, top2)
    nc = _get_program(C)

    if trace:
        _maybe_install_trace_shim()
    res = run_bass_kernel_spmd(
        nc, in_maps, core_ids=list(range(E)), trace=trace,
        trace_cores=list(range(E)) if trace else None, stitch_traces=False,
    )

    out2d = np.zeros((B * S, D), np.float32)
    usage_sum = np.zeros(E, np.float32)
    z_sum = np.float32(0.0)
    for e in range(E):
        r = res.results[e]
        idx = lists[e]
        out2d[idx] += r["yg"][: len(idx)]
        usage_sum += r["partials"][0, :E]
        z_sum += r["partials"][0, E]

    n_tok = np.float32(B * S)
    usage = usage_sum * np.float32(0.5) / n_tok
    mu = usage.mean(dtype=np.float32)
    var = ((usage - mu) ** 2).mean(dtype=np.float32)
    lb = var / (mu * mu + np.float32(1e-8)) * np.float32(E) * np.float32(LB_W)
    z = np.float32(0.5) * z_sum / n_tok * np.float32(Z_W)

    out = out2d.reshape(B, S, D)
    return (out, np.float32(lb), np.float32(z)), res


def kernel(**inputs):
    result, _ = run_moe(inputs, trace=bool(int(os.environ.get("MOE_TRACE", "0"))))
    return result


# revision 12
# speedup vs baseline: 1.1943x; 1.0287x over previous
"""MoE MiniGPT block kernel for 8 TRN2 NeuronCores (expert-parallel, top-2 sparse).

Strategy:
  - Host computes only integer routing decisions (which tokens are in each
    expert's top-2 set) and reshapes/shards data. All output-value arithmetic
    (router softmax, top-2 renormalized combine weights, FFN with exact GELU,
    aux-loss token statistics) runs on device.
  - Core e receives expert e's weights plus the tokens routed to it (padded to
    a shared capacity C, transposed to [D, C] for the matmul layout).
  - Device per core: router logits -> softmax -> combine weight for this
    expert; FFN y = (gelu(x@w1+b1) @ w2 + b2) * combine; per-core partial sums
    for load-balancing usage and router-z loss (each token is counted on
    exactly its TOP_K=2 cores, so host scales the summed partials by 0.5).
  - Host scatter-adds the two per-expert outputs per token and assembles the
    two scalar losses from device partials.

Matmuls run as float32r (full PE rate); the tiny router/reduction matmuls run
as float32 for precision.
"""

import os

import numpy as np

import concourse.bass as bass
import concourse.mybir as mybir
import concourse.tile as tile
from concourse import bacc
from concourse.bass_utils import run_bass_kernel_spmd

F32 = mybir.dt.float32
F32R = mybir.dt.float32r

E = 8
TOP_K = 2
D = 768
F = 3072
KD = D // 128  # 6
KF = F // 128  # 24
TB = 384  # token block (3 sub-tiles of 128)
LB_W = 0.01
Z_W = 0.001

AF = mybir.ActivationFunctionType
ALU = mybir.AluOpType
AX = mybir.AxisListType

_MM_DT = F32R if os.environ.get("MOE_MM_DT", "f32r") == "f32r" else F32


def build_moe_program(C: int):
    """Trace the SPMD Bass/Tile program for capacity C (multiple of TB)."""
    assert C % TB == 0
    nts = C // 128
    nb = C // TB
    WG = 4  # weight DMA split (per-queue pipelining granularity)

    nc = bacc.Bacc(None, target_bir_lowering=False)

    # Host-prepared layouts: per-SBUF-partition data is contiguous in DRAM so
    # each DMA descriptor covers multi-KB runs (packet-overhead-bound
    # otherwise). xgp[b, p, k, t] = x_gathered[b*TB+t, k*128+p].
    xgp = nc.dram_tensor("xgp", [nb, 128, KD, TB], _MM_DT, kind="ExternalInput")
    Wr = nc.dram_tensor("Wr", [D, E], _MM_DT, kind="ExternalInput")
    w1p = nc.dram_tensor("w1p", [WG, 128, KD, F // WG], _MM_DT, kind="ExternalInput")
    b1T = nc.dram_tensor("b1T", [128, KF], F32, kind="ExternalInput")
    w2p = nc.dram_tensor("w2p", [WG, 128, KF // WG, D], _MM_DT, kind="ExternalInput")
    b2c = nc.dram_tensor("b2c", [128, D], F32, kind="ExternalInput")
    sel = nc.dram_tensor("sel", [128, E], F32, kind="ExternalInput")
    maskc = nc.dram_tensor("maskc", [128, nts], F32, kind="ExternalInput")
    onescol = nc.dram_tensor("onescol", [128, 1], F32, kind="ExternalInput")
    yg = nc.dram_tensor("yg", [C, D], F32, kind="ExternalOutput")
    partials = nc.dram_tensor("partials", [1, E + 1], F32, kind="ExternalOutput")

    with tile.TileContext(nc) as tc:
        with (
            tc.tile_pool(name="weights", bufs=1) as wp,
            tc.tile_pool(name="consts", bufs=1) as cp,
            tc.tile_pool(name="p1s", bufs=4) as p1s,
            tc.tile_pool(name="p1w", bufs=4) as p1w,
            tc.tile_pool(name="p2x", bufs=2) as p2x,
            tc.tile_pool(name="p2h", bufs=3) as p2h,
            tc.tile_pool(name="p2y", bufs=3) as p2y,
            tc.tile_pool(name="psh", bufs=2, space="PSUM") as psh,
            tc.tile_pool(name="psy", bufs=3, space="PSUM") as psy,
        ):
            # Weight loads: w1 on the gpsimd SWDGE queue, w2 on the scalar
            # HWDGE queue, in groups so the FFN can start on group 0 early.
            # Everything else streams on the sync queue.
            w1_sb = wp.tile([128, KD, F], _MM_DT, tag="w1")
            for g in range(WG):
                gs = F // WG
                nc.gpsimd.dma_start(w1_sb[:, :, g * gs:(g + 1) * gs], w1p[g])
            w2_sb = wp.tile([128, KF, D], _MM_DT, tag="w2")
            for g in range(WG):
                gs = KF // WG
                nc.scalar.dma_start(w2_sb[:, g * gs:(g + 1) * gs, :], w2p[g])

            wr_sb = cp.tile([128, KD, E], _MM_DT, tag="wr")
            nc.sync.dma_start(wr_sb[:], Wr[:].rearrange("(k p) e -> p k e", p=128))
            b1_sb = cp.tile([128, KF], F32, tag="b1")
            nc.sync.dma_start(b1_sb[:], b1T[:])
            b2_sb = cp.tile([128, D], F32, tag="b2")
            nc.sync.dma_start(b2_sb[:], b2c[:])
            sel_sb = cp.tile([128, E], F32, tag="sel")
            nc.sync.dma_start(sel_sb[:], sel[:])
            mask_sb = cp.tile([128, nts], F32, tag="mask")
            nc.sync.dma_start(mask_sb[:], maskc[:])
            ones_c = cp.tile([128, 1], F32, tag="onesc")
            nc.sync.dma_start(ones_c[:], onescol[:])
            comb_all = cp.tile([128, nts], F32, tag="comb")
            ucols = cp.tile([128, nts, E], F32, tag="ucols")
            scols = cp.tile([128, nts], F32, tag="scols")
            ncols = cp.tile([128, nts], F32, tag="ncols")
            stag = cp.tile([128, E + 1], F32, tag="stag")

            for b in range(nb):
                xb = p2x.tile([128, KD, TB], _MM_DT, tag="xb")
                nc.sync.dma_start(xb[:], xgp[b])

                # router / combine / loss stats for this block's 3 sub-tiles
                for tsl in range(3):
                    ts = b * 3 + tsl
                    lg = psh.tile([128, E], F32, tag="ph", name=f"lg{ts}")
                    for k in range(KD):
                        nc.tensor.matmul(
                            lg[:],
                            lhsT=xb[:, k, tsl * 128:(tsl + 1) * 128],
                            rhs=wr_sb[:, k, :],
                            start=(k == 0),
                            stop=(k == KD - 1),
                        )
                    negmax = ncols[:, ts:ts + 1]
                    nc.vector.tensor_reduce(
                        negmax, lg[:], axis=AX.X, op=ALU.max, negate=True
                    )
                    expt = p1w.tile([128, E], F32, tag="expt")
                    nc.scalar.activation(
                        expt[:], lg[:], AF.Exp, bias=negmax,
                        accum_out=scols[:, ts:ts + 1],
                    )
                    rS = p1s.tile([128, 1], F32, tag="rS")
                    nc.vector.reciprocal(rS[:], scols[:, ts:ts + 1])
                    probs = p1w.tile([128, E], F32, tag="probs")
                    nc.vector.tensor_scalar_mul(probs[:], expt[:], rS[:])
                    m1 = p1s.tile([128, 1], F32, tag="m1")
                    nc.vector.tensor_reduce(m1[:], probs[:], axis=AX.X, op=ALU.max)
                    istop = p1w.tile([128, E], F32, tag="istop")
                    nc.vector.tensor_scalar(
                        istop[:], probs[:], m1[:], None, op0=ALU.is_ge
                    )
                    masked = p1w.tile([128, E], F32, tag="masked")
                    nc.vector.scalar_tensor_tensor(
                        masked[:], istop[:], -1e9, probs[:], op0=ALU.mult, op1=ALU.add
                    )
                    m2 = p1s.tile([128, 1], F32, tag="m2")
                    nc.vector.tensor_reduce(m2[:], masked[:], axis=AX.X, op=ALU.max)
                    den = p1s.tile([128, 1], F32, tag="den")
                    nc.vector.tensor_tensor(den[:], m1[:], m2[:], op=ALU.add)
                    den2 = p1s.tile([128, 1], F32, tag="den2")
                    nc.vector.tensor_scalar_add(den2[:], den[:], 1e-8)
                    rden = p1s.tile([128, 1], F32, tag="rden")
                    nc.vector.reciprocal(rden[:], den2[:])
                    psel_t = p1w.tile([128, E], F32, tag="pselt")
                    psel = p1s.tile([128, 1], F32, tag="psel")
                    nc.vector.tensor_tensor(psel_t[:], probs[:], sel_sb[:], op=ALU.mult)
                    nc.vector.tensor_reduce(psel[:], psel_t[:], axis=AX.X, op=ALU.add)
                    nc.vector.tensor_tensor(
                        comb_all[:, ts:ts + 1], psel[:], rden[:], op=ALU.mult
                    )
                    nc.vector.tensor_scalar_mul(
                        ucols[:, ts, :], probs[:], mask_sb[:, ts:ts + 1]
                    )

                # FFN
                py = []
                for ts in range(3):
                    py.append(psy.tile([128, D], F32, tag="py", name=f"py{ts}"))
                for f in range(KF):
                    ph = psh.tile([128, TB], F32, tag="ph")
                    for k in range(KD):
                        nc.tensor.matmul(
                            ph[:],
                            lhsT=(w1_sb[:, k, f * 128:(f + 1) * 128]),
                            rhs=(xb[:, k, :]),
                            start=(k == 0),
                            stop=(k == KD - 1),
                        )
                    hs = p2h.tile([128, TB], _MM_DT, tag="hs")
                    nc.scalar.activation(
                        hs[:], ph[:], AF.Gelu, bias=b1_sb[:, f:f + 1]
                    )
                    for ts in range(3):
                        nc.tensor.matmul(
                            py[ts][:, 0:512],
                            lhsT=(hs[:, ts * 128:(ts + 1) * 128]),
                            rhs=(w2_sb[:, f, 0:512]),
                            start=(f == 0),
                            stop=(f == KF - 1),
                        )
                        nc.tensor.matmul(
                            py[ts][:, 512:D],
                            lhsT=(hs[:, ts * 128:(ts + 1) * 128]),
                            rhs=(w2_sb[:, f, 512:D]),
                            start=(f == 0),
                            stop=(f == KF - 1),
                        )
                for ts in range(3):
                    gts = b * 3 + ts
                    # evict: ysb = (py + b2) * combine, on DVE (frees ACT)
                    ysum = p2y.tile([128, D], F32, tag="ysum")
                    nc.vector.tensor_tensor(ysum[:], py[ts][:], b2_sb[:], op=ALU.add)
                    ysb = p2y.tile([128, D], F32, tag="ysb")
                    nc.vector.tensor_scalar_mul(
                        ysb[:], ysum[:], comb_all[:, gts:gts + 1]
                    )
                    nc.sync.dma_start(
                        yg[b * TB + ts * 128:b * TB + (ts + 1) * 128, :], ysb[:]
                    )

            # loss partials: z-columns batched, then partition-reduce
            lnall = p1w.tile([128, nts], F32, tag="lnall")
            nc.scalar.activation(lnall[:], scols[:], AF.Ln)
            lseall = p1w.tile([128, nts], F32, tag="lseall")
            nc.vector.tensor_tensor(lseall[:], lnall[:], ncols[:], op=ALU.subtract)
            lsqall = p1w.tile([128, nts], F32, tag="lsqall")
            nc.vector.tensor_tensor(lsqall[:], lseall[:], lseall[:], op=ALU.mult)
            zcols = p1w.tile([128, nts], F32, tag="zcols")
            nc.vector.tensor_tensor(zcols[:], lsqall[:], mask_sb[:], op=ALU.mult)
            nc.vector.tensor_reduce(
                stag[:, 0:E],
                ucols[:].rearrange("p t e -> p e t"),
                axis=AX.X,
                op=ALU.add,
            )
            nc.vector.tensor_reduce(stag[:, E:E + 1], zcols[:], axis=AX.X, op=ALU.add)
            pred = psh.tile([1, E + 1], F32, tag="ph", name="pred")
            nc.tensor.matmul(pred[:], lhsT=ones_c[:], rhs=stag[:], start=True, stop=True)
            part_sb = p1s.tile([1, E + 1], F32, tag="partsb")
            nc.scalar.copy(part_sb[:], pred[:])
            nc.sync.dma_start(partials[:], part_sb[:])
    nc.compile()
    return nc


# ---------------------------------------------------------------------------
# Host side
# ---------------------------------------------------------------------------

def _route(x2d: np.ndarray, Wr: np.ndarray):
    """Top-2 routing decisions (integer only; values recomputed on device)."""
    logits = x2d @ Wr  # [N, E] fp32
    lmax = logits.max(-1, keepdims=True)
    p = np.exp(logits - lmax)
    p /= p.sum(-1, keepdims=True)
    top2 = np.argsort(-p, axis=-1, kind="stable")[:, :TOP_K]  # [N, 2]
    return top2


def _prep_core_inputs(x2d, Wr, w1, b1, w2, b2, top2):
    n_tok = x2d.shape[0]
    lists = [np.nonzero((top2 == e).any(-1))[0] for e in range(E)]
    cmax = max(len(l) for l in lists)
    C = max(TB, ((cmax + TB - 1) // TB) * TB)
    nts = C // 128
    in_maps = []
    for e in range(E):
        idx = lists[e]
        cnt = len(idx)
        xg = np.zeros((C, D), np.float32)
        xg[:cnt] = x2d[idx]
        nb = C // TB
        WG = 4
        xgp = np.ascontiguousarray(
            xg.reshape(nb, TB, KD, 128).transpose(0, 3, 2, 1)
        )
        w1p = np.ascontiguousarray(
            w1[e].reshape(KD, 128, WG, F // WG).transpose(2, 1, 0, 3)
        )
        w2p = np.ascontiguousarray(
            w2[e].reshape(WG, KF // WG, 128, D).transpose(0, 2, 1, 3)
        )
        mask = np.zeros((128, nts), np.float32)
        flat = np.arange(C) < cnt
        mask[:, :] = flat.reshape(nts, 128).T
        sel = np.zeros((128, E), np.float32)
        sel[:, e] = 1.0
        in_maps.append({
            "xgp": xgp,
            "Wr": np.ascontiguousarray(Wr),
            "w1p": w1p,
            "b1T": np.ascontiguousarray(b1[e].reshape(KF, 128).T),
            "w2p": w2p,
            "b2c": np.ascontiguousarray(np.broadcast_to(b2[e], (128, D))),
            "sel": sel,
            "maskc": mask,
            "onescol": np.ones((128, 1), np.float32),
        })
    return in_maps, lists, C


_PROGRAM_CACHE: dict[int, object] = {}


def _get_program(C: int):
    if os.environ.get("MOE_PHASE"):
        return build_moe_program(C)
    if C not in _PROGRAM_CACHE:
        _PROGRAM_CACHE[C] = build_moe_program(C)
    return _PROGRAM_CACHE[C]


def _maybe_install_trace_shim():
    """antenv.axon_hooks is absent in this image; recreate it from trn_boot so
    trace=True can capture NTFF profiles through the axon .so."""
    import sys
    import types

    if "antenv.axon_hooks" in sys.modules:
        return
    try:
        from trn_agent_boot.trn_boot import _ntff_profile_via_ctypes

        hook = _ntff_profile_via_ctypes("/opt/axon/libaxon_pjrt.so")
    except Exception:
        return
    mod = types.ModuleType("antenv.axon_hooks")
    mod.get_axon_ntff_profile_hook = lambda: hook
    mod.set_axon_ntff_profile_hook = lambda h: None
    sys.modules["antenv.axon_hooks"] = mod


def run_moe(inputs: dict, trace: bool = False):
    """Run the kernel; returns ((out, lb_loss, z_loss), BassKernelResults)."""
    x = np.asarray(inputs["x"], np.float32)
    Wr = np.asarray(inputs["Wr"], np.float32)
    w1 = np.asarray(inputs["w1"], np.float32)
    b1 = np.asarray(inputs["b1"], np.float32)
    w2 = np.asarray(inputs["w2"], np.float32)
    b2 = np.asarray(inputs["b2"], np.float32)

    B, S, _ = x.shape
    x2d = np.ascontiguousarray(x.reshape(B * S, D))
    top2 = _route(x2d, Wr)
    in_maps, lists, C = _prep_core_inputs(x2d, Wr, w1, b1, w2, b2, top2)
    nc = _get_program(C)

    if trace:
        _maybe_install_trace_shim()
    res = run_bass_kernel_spmd(
        nc, in_maps, core_ids=list(range(E)), trace=trace,
        trace_cores=list(range(E)) if trace else None, stitch_traces=False,
    )

    out2d = np.zeros((B * S, D), np.float32)
    usage_sum = np.zeros(E, np.float32)
    z_sum = np.float32(0.0)
    for e in range(E):
        r = res.results[e]
        idx = lists[e]
        out2d[idx] += r["yg"][: len(idx)]
        usage_sum += r["partials"][0, :E]
        z_sum += r["partials"][0, E]

    n_tok = np.float32(B * S)
    usage = usage_sum * np.float32(0.5) / n_tok
    mu = usage.mean(dtype=np.float32)
    var = ((usage - mu) ** 2).mean(dtype=np.float32)
    lb = var / (mu * mu + np.float32(1e-8)) * np.float32(E) * np.float32(LB_W)
    z = np.float32(0.5) * z_sum / n_tok * np.float32(Z_W)

    out = out2d.reshape(B, S, D)
    return (out, np.float32(lb), np.float32(z)), res


def kernel(**inputs):
    result, _ = run_moe(inputs, trace=bool(int(os.environ.get("MOE_TRACE", "0"))))
    return result
